# revision 1
# baseline (speedup 1.0000x reference)
"""Trainium2 Bass kernel for EquivariantMPLayer (GNN message passing).

  msg_repr = [x[row], x[col], edge_dist]            # [E, 2C+1]
  messages = relu(msg_repr @ W_msg + b_msg)         # [E, H]
  aggr     = segment_sum(messages, col, N)          # [N, H]
  out      = x @ W_res + relu([x, aggr] @ W_upd + b_upd)

Strategy (8 NeuronCores, SPMD single program):
  * Host: sort edges by col; shard cores by contiguous node ranges, so each
    core's local segment-sum is the complete aggregate for its node slice --
    no cross-core reduction at all. Within a core, nodes are split into
    variable-width blocks (<=126 nodes, <=T*128 edges) so edge tiles are
    ~95% full; every block gets exactly T tiles (uniform SPMD stream).
  * Per edge tile (128 edges): indirect-DMA gather x[row] (the only
    per-edge random access; SWDGE descriptor-generation bound), then on PE:
      pre[e,H] = x_row @ W1  +  bt^T @ c_aug
    where bt[e, 0:126] is the one-hot col indicator (one DVE is-equal
    against an iota constant), bt[e,126]=valid, bt[e,127]=dist, and
    c_aug = [x_block @ W2 ; b_msg ; w3]. One matmul applies the col-side
    message term, the bias, and the dist*w3 term at once.
      msg = relu(pre)                   (ACT)
      aggr_block += bt[:,0:126]^T @ msg (PE, PSUM accumulation over tiles)
    Stationary matmul operands are bf16 (2x faster LDWEIGHTS).
  * Node update per block is a few 128x128 matmuls against the kept x^T.
"""
import numpy as np
import ml_dtypes
import os
BF16 = bool(int(os.environ.get("K_BF16", "1")))

N = 50000
E = 800000
C = 128
H = 128
NCORES = 8
BLK = 126                    # max nodes per block
TB = 16                      # tiles per block (uniform)
ECAP = TB * 128              # max edges per block
NODES_PER_CORE = 6300        # fixed contiguous node range per core


def _build_and_run(in_maps, NB):
    import concourse.bacc as bacc
    import concourse.tile as tile
    from concourse import bass, mybir
    from concourse.bass_utils import run_bass_kernel_spmd

    f32 = mybir.dt.float32
    bf16 = mybir.dt.bfloat16 if BF16 else mybir.dt.float32
    i32 = mybir.dt.int32
    P = 128
    T = TB

    nc = bacc.Bacc("TRN2")
    nc.cache_partition_id()

    node_embed = nc.dram_tensor("node_embed", [N, C], f32, kind="ExternalInput")
    x_blocks = nc.dram_tensor("x_blocks", [NB, P, C], f32, kind="ExternalInput")
    gidx = nc.dram_tensor("gidx", [NB, P, T], i32, kind="ExternalInput")
    colmod = nc.dram_tensor("colmod", [NB, P, T], f32, kind="ExternalInput")
    tail = nc.dram_tensor("tail", [NB, P, 2 * T], f32, kind="ExternalInput")
    cmrows = nc.dram_tensor("cmrows", [NB, T, P], f32, kind="ExternalInput")
    tailrows = nc.dram_tensor("tailrows", [NB, T, 2, P], bf16, kind="ExternalInput")
    iotap_d = nc.dram_tensor("iotap", [P, P], f32, kind="ExternalInput")
    W1 = nc.dram_tensor("W1", [C, H], bf16, kind="ExternalInput")
    W2 = nc.dram_tensor("W2", [C, H], f32, kind="ExternalInput")
    bmsg_w3 = nc.dram_tensor("bmsg_w3", [2, H], bf16, kind="ExternalInput")
    Wu1 = nc.dram_tensor("Wu1", [C, H], f32, kind="ExternalInput")
    Wu2 = nc.dram_tensor("Wu2", [H, H], f32, kind="ExternalInput")
    Wres = nc.dram_tensor("Wres", [C, H], f32, kind="ExternalInput")
    bupd = nc.dram_tensor("bupd", [H, 1], f32, kind="ExternalInput")
    iota = nc.dram_tensor("iota", [P, P], f32, kind="ExternalInput")
    ident = nc.dram_tensor("ident", [P, P], f32, kind="ExternalInput")
    identb = nc.dram_tensor("identb", [P, P], bf16, kind="ExternalInput")
    out_d = nc.dram_tensor("out", [NB * BLK, H], f32, kind="ExternalOutput")

    RELU = mybir.ActivationFunctionType.Relu
    EQ = mybir.AluOpType.is_equal

    with tile.TileContext(nc) as tc:
        with tc.tile_pool(name="const", bufs=1) as cp, \
             tc.tile_pool(name="persist", bufs=1) as pp, \
             tc.tile_pool(name="work", bufs=8) as wp, \
             tc.tile_pool(name="xg", bufs=16) as gp, \
             tc.tile_pool(name="xgd", bufs=16) as gdp, \
             tc.tile_pool(name="psum3", bufs=2, space="PSUM") as ps, \
             tc.tile_pool(name="psum2", bufs=2, space="PSUM") as ps2:

            def load_const(t, name):
                tl = cp.tile(list(t.shape), t.dtype, tag=name)
                nc.sync.dma_start(out=tl[:], in_=t[:])
                return tl

            w1 = load_const(W1, "w1")
            w2 = load_const(W2, "w2")
            wu1 = load_const(Wu1, "wu1")
            wu2 = load_const(Wu2, "wu2")
            wres = load_const(Wres, "wres")
            bu = load_const(bupd, "bu")
            io_t = load_const(iota, "iota")
            io_p = load_const(iotap_d, "iotap")
            idt = load_const(ident, "ident")
            idtb = load_const(identb, "identb")

            # ---------- phase 0: preload all per-block edge metadata ----------
            gixA, cmodA, tailA = [], [], []
            for b in range(NB):
                gix = pp.tile([P, T], i32, tag=f"gix{b}")
                nc.sync.dma_start(out=gix[:], in_=gidx[b])
                cmod = pp.tile([P, T], f32, tag=f"cmod{b}")
                nc.sync.dma_start(out=cmod[:], in_=colmod[b])
                tl = pp.tile([P, 2 * T], f32, tag=f"tail{b}")
                nc.sync.dma_start(out=tl[:], in_=tail[b])
                gixA.append(gix)
                cmodA.append(cmod)
                tailA.append(tl)

            # ---------- phase C: per-block x^T and c_aug ----------
            xT = []
            caug = []
            for b in range(NB):
                xb = wp.tile([P, C], f32, tag="xb")
                nc.sync.dma_start(out=xb[:], in_=x_blocks[b])
                ptx = ps.tile([P, P], f32, space="PSUM", tag="ptx")
                nc.tensor.transpose(out=ptx[:], in_=xb[:], identity=idt[:])
                xt = pp.tile([C, P], f32, tag=f"xT{b}")
                nc.scalar.copy(out=xt[:], in_=ptx[:])
                pc = ps.tile([P, H], f32, space="PSUM", tag="pre")
                nc.tensor.matmul(out=pc[:], lhsT=xt[:], rhs=w2[:], start=True, stop=True)
                ca = pp.tile([P, H], bf16, tag=f"caug{b}")
                nc.vector.tensor_copy(out=ca[0:BLK, :], in_=pc[0:BLK, :])
                nc.sync.dma_start(out=ca[BLK:P, :], in_=bmsg_w3[:])
                xT.append(xt)
                caug.append(ca)

            # ---------- phase E: edges; phase U: node update ----------
            for b in range(NB):
                gix = gixA[b]
                cmod = cmodA[b]
                tl = tailA[b]

                pagg = ps2.tile([P, H], f32, space="PSUM", tag="agg")
                for t in range(T):
                    xg = gdp.tile([P, C], f32, tag="xg")
                    nc.gpsimd.indirect_dma_start(
                        out=xg[:], out_offset=None, in_=node_embed[:],
                        in_offset=bass.IndirectOffsetOnAxis(ap=gix[:, t:t + 1], axis=0))
                    ptx = ps.tile([P, P], f32, space="PSUM", tag="ptx")
                    nc.tensor.transpose(out=ptx[:], in_=xg[:], identity=idt[:])
                    xts = wp.tile([C, P], bf16, tag="xts")
                    nc.vector.tensor_copy(out=xts[:], in_=ptx[:])

                    bt = gp.tile([P, P], bf16, tag="bt")
                    nc.vector.tensor_scalar(bt[:], io_t[:], cmod[:, t:t + 1], None, EQ)
                    nc.vector.tensor_copy(out=bt[:, BLK:P], in_=tl[:, 2 * t:2 * t + 2])
                    pbt = ps2.tile([P, P], bf16, space="PSUM", tag="pbt")
                    nc.tensor.transpose(out=pbt[:], in_=bt[:], identity=idtb[:])
                    btT = wp.tile([P, P], bf16, tag="btT")
                    nc.vector.tensor_copy(out=btT[:], in_=pbt[:])

                    ppre = ps.tile([P, H], f32, space="PSUM", tag="pre")
                    nc.tensor.matmul(out=ppre[:], lhsT=xts[:], rhs=w1[:], start=True, stop=False)
                    nc.tensor.matmul(out=ppre[:], lhsT=btT[:], rhs=caug[b][:], start=False, stop=True)
                    msg = wp.tile([P, H], bf16, tag="msg")
                    nc.scalar.activation(out=msg[:], in_=ppre[:], func=RELU)
                    nc.tensor.matmul(out=pagg[0:BLK, :], lhsT=bt[:, 0:BLK], rhs=msg[:],
                                     start=(t == 0), stop=(t == T - 1))

                # ----- node update for block b -----
                aggs = wp.tile([P, H], f32, tag="aggs")
                nc.vector.memset(aggs[:], 0)
                nc.vector.tensor_copy(out=aggs[0:BLK, :], in_=pagg[0:BLK, :])
                pat = ps.tile([P, P], f32, space="PSUM", tag="ptx")
                nc.tensor.transpose(out=pat[:], in_=aggs[:], identity=idt[:])
                aggT = wp.tile([H, P], f32, tag="aggT")
                nc.vector.tensor_copy(out=aggT[:], in_=pat[:])

                pupd = ps.tile([H, P], f32, space="PSUM", tag="pre")
                nc.tensor.matmul(out=pupd[:], lhsT=wu1[:], rhs=xT[b][:], start=True, stop=False)
                nc.tensor.matmul(out=pupd[:], lhsT=wu2[:], rhs=aggT[:], start=False, stop=True)
                rel = wp.tile([H, P], f32, tag="rel")
                nc.scalar.activation(out=rel[:], in_=pupd[:], func=RELU, bias=bu[:])

                pout = ps.tile([H, P], f32, space="PSUM", tag="ptx")
                nc.tensor.matmul(out=pout[:], lhsT=wres[:], rhs=xT[b][:], start=True, stop=True)
                outT = wp.tile([H, P], f32, tag="outT")
                nc.vector.tensor_tensor(out=outT[:], in0=pout[:], in1=rel[:],
                                        op=mybir.AluOpType.add)
                pfin = ps.tile([P, H], f32, space="PSUM", tag="ptx")
                nc.tensor.transpose(out=pfin[:], in_=outT[:], identity=idt[:])
                outs = wp.tile([P, H], f32, tag="outs")
                nc.scalar.copy(out=outs[:], in_=pfin[:])
                nc.sync.dma_start(out=out_d[b * BLK:(b + 1) * BLK, :], in_=outs[0:BLK, :])

    nc.finalize()
    res = run_bass_kernel_spmd(nc, in_maps, core_ids=list(range(NCORES)),
                               trace=bool(int(__import__("os").environ.get("K_TRACE", "0"))))
    return res


def kernel(node_embed, edge_dist, edge_index, W_res, W_msg, b_msg, W_upd, b_upd):
    node_embed = np.asarray(node_embed, dtype=np.float32)
    edge_dist = np.asarray(edge_dist, dtype=np.float32).reshape(-1)
    row = np.asarray(edge_index[0], dtype=np.int64).astype(np.int32)
    col = np.asarray(edge_index[1], dtype=np.int64).astype(np.int32)
    W_res = np.asarray(W_res, dtype=np.float32)
    W_msg = np.asarray(W_msg, dtype=np.float32)
    b_msg = np.asarray(b_msg, dtype=np.float32)
    W_upd = np.asarray(W_upd, dtype=np.float32)
    b_upd = np.asarray(b_upd, dtype=np.float32)

    order = np.argsort(col, kind="stable")
    scol = col[order]
    srow = row[order]
    sdist = edge_dist[order]

    # per-core greedy blocks: <=BLK nodes, <=ECAP edges
    core_blocks = []   # per core: list of (node_start, node_end, e0, e1)
    for core in range(NCORES):
        n0 = core * NODES_PER_CORE
        n1 = min(n0 + NODES_PER_CORE, N)
        blocks = []
        v = n0
        while v < n1:
            vmax = min(v + BLK, n1)
            e0 = np.searchsorted(scol, v)
            emax = np.searchsorted(scol, vmax)
            if emax - e0 <= ECAP:
                vend = vmax
                e1 = emax
            else:
                # find largest vend with edge count <= ECAP
                e1 = e0 + ECAP
                vend = int(scol[e1 - 1])  # last fully-included node candidate
                # all edges of node vend must fit; back off to node boundary
                e1 = np.searchsorted(scol, vend)
                vend = max(vend, v + 1)
                e1 = np.searchsorted(scol, vend)
            blocks.append((v, vend, int(e0), int(e1)))
            v = vend
        core_blocks.append(blocks)

    NB = max(len(b) for b in core_blocks)
    P = 128
    T = TB
    gidx = np.zeros((NCORES, NB, P, T), np.int32)
    colm = np.full((NCORES, NB, P, T), -1.0, np.float32)
    tailh = np.zeros((NCORES, NB, P, 2 * T), np.float32)
    x_blocks = np.zeros((NCORES, NB, P, C), np.float32)

    for core in range(NCORES):
        for b, (v0, v1, e0, e1) in enumerate(core_blocks[core]):
            cnt = e1 - e0
            if cnt:
                idx = np.arange(cnt)
                tt, pp_ = idx // 128, idx % 128
                gidx[core, b, pp_, tt] = srow[e0:e1]
                colm[core, b, pp_, tt] = (scol[e0:e1] - v0).astype(np.float32)
                tailh[core, b, pp_, 2 * tt] = 1.0
                tailh[core, b, pp_, 2 * tt + 1] = sdist[e0:e1]
            x_blocks[core, b, 0:v1 - v0, :] = node_embed[v0:v1]

    iota = np.tile(np.arange(P, dtype=np.float32), (P, 1))
    iota[:, BLK:] = -5.0
    iotap = np.repeat(np.arange(P, dtype=np.float32)[:, None], P, axis=1)
    iotap[BLK:, :] = -6.0
    cmrows = np.ascontiguousarray(colm.transpose(0, 1, 3, 2))
    tailrows = np.ascontiguousarray(
        tailh.reshape(NCORES, NB, P, T, 2).transpose(0, 1, 3, 4, 2)
    ).astype(ml_dtypes.bfloat16 if BF16 else np.float32)
    consts = {
        "W1": W_msg[0:C].astype(ml_dtypes.bfloat16 if BF16 else np.float32),
        "W2": W_msg[C:2 * C],
        "bmsg_w3": np.stack([b_msg, W_msg[2 * C]]).astype(ml_dtypes.bfloat16 if BF16 else np.float32),
        "Wu1": W_upd[0:C], "Wu2": W_upd[C:C + H],
        "Wres": W_res, "bupd": b_upd.reshape(H, 1),
        "iota": iota, "iotap": iotap, "ident": np.eye(P, dtype=np.float32),
        "identb": np.eye(P).astype(ml_dtypes.bfloat16 if BF16 else np.float32),
    }
    in_maps = []
    for core in range(NCORES):
        m = {"node_embed": node_embed, "x_blocks": x_blocks[core],
             "gidx": gidx[core], "colmod": colm[core], "tail": tailh[core],
             "cmrows": cmrows[core], "tailrows": tailrows[core]}
        m.update(consts)
        in_maps.append(m)

    res = _build_and_run(in_maps, NB)
    kernel._last_result = res

    out = np.empty((N, H), np.float32)
    for core in range(NCORES):
        o = res.results[core]["out"]
        for b, (v0, v1, _, _) in enumerate(core_blocks[core]):
            out[v0:v1] = o[b * BLK:b * BLK + (v1 - v0)]
    return out



# revision 3
# speedup vs baseline: 1.1978x; 1.1978x over previous
"""Trainium2 Bass kernel for EquivariantMPLayer (GNN message passing), v3.

  msg_repr = [x[row], x[col], edge_dist]            # [E, 2C+1]
  messages = relu(msg_repr @ W_msg + b_msg)         # [E, H]
  aggr     = segment_sum(messages, col, N)          # [N, H]
  out      = x @ W_res + relu([x, aggr] @ W_upd + b_upd)

Strategy (8 NeuronCores, SPMD single program):
  * Host: sort edges by col; shard cores by contiguous node ranges so each
    core's local segment-sum is complete for its node slice. Blocks of
    <=126 nodes; edges split into <=1024 low-row (<25000) and <=1024
    high-row slots; 16 tiles of 128 edges.
  * Phase Y1 (replicated): Y1 = x @ W1 (bf16 -> Internal DRAM), batched 4
    tiles per DVE/DMA op. b_msg rides in caug row 126 against the one-hot
    "valid" row, so no bias pass is needed.
  * Phase E per block: two 1024-row SWDGE dma_gathers of Y1 (row-side
    term; >1024 idxs per instruction deadlocks, and descgen costs ~9ns/row
    on the Q7 so the gathers dominate gpsimd time; queue_num rotation
    spreads them across Q7 core pairs when enabled). One-hot matrices come
    from the host in fp8 (btT8 includes valid row 126 and dist row 127),
    removing all per-tile is_equal/transpose/copyout work. Per 4-tile
    group: 4 matmuls btT8^T @ caug + one fused DVE (ppre + y1g); one ACT
    relu per block; per tile one aggregation matmul msg^T @ bt8
    accumulating aggT [H, node] in PSUM.
  * Node update per block in [H, node] orientation; PE transpose for the
    output write.
"""
import numpy as np
import ml_dtypes
import os

N = 50000
E = 800000
C = 128
H = 128
NCORES = 8
BLK = 126                    # max nodes per block
TB = 16                      # tiles per block
HCAP = 1024                  # per-block capacity of low/high-row halves
NSPLIT = 25000               # row id splitting low/high gather tables
NODES_PER_CORE = 6300
NT = (N + 127) // 128        # Y1 phase tiles (391)
P = 128
QROT = int(os.environ.get("K_QROT", "1"))   # rotate gather queue_num 0..3


def _build_and_run(in_maps, NB):
    import concourse.bacc as bacc
    import concourse.tile as tile
    from concourse import bass, library_config, mybir
    from concourse.bass_utils import run_bass_kernel_spmd

    f32 = mybir.dt.float32
    bf16 = mybir.dt.bfloat16
    fp8 = mybir.dt.float8e4
    i16 = mybir.dt.int16
    T = TB
    IW = HCAP // 16          # idx columns per gather instruction (64)

    nc = bacc.Bacc("TRN2", num_swdge_queues=4 if QROT else 1)
    nc.cache_partition_id()

    xT = nc.dram_tensor("xT", [C, N], bf16, kind="ExternalInput")
    xTc = nc.dram_tensor("xTc", [C, NB * BLK], bf16, kind="ExternalInput")
    idx_d = nc.dram_tensor("idxq", [P, NB * 2 * IW], i16, kind="ExternalInput")
    btT8_d = nc.dram_tensor("btT8", [P, NB * T * P], fp8, kind="ExternalInput")
    bt8_d = nc.dram_tensor("bt8", [P, NB * T * BLK], fp8, kind="ExternalInput")
    W1 = nc.dram_tensor("W1", [C, H], bf16, kind="ExternalInput")
    W2 = nc.dram_tensor("W2", [C, H], bf16, kind="ExternalInput")
    Wu1 = nc.dram_tensor("Wu1", [C, H], bf16, kind="ExternalInput")
    Wu2 = nc.dram_tensor("Wu2", [H, H], bf16, kind="ExternalInput")
    Wres = nc.dram_tensor("Wres", [C, H], bf16, kind="ExternalInput")
    bupd = nc.dram_tensor("bupd", [H, 1], f32, kind="ExternalInput")
    c2_d = nc.dram_tensor("c2", [2, H], bf16, kind="ExternalInput")
    identf_d = nc.dram_tensor("identf", [P, P], f32, kind="ExternalInput")
    Y1 = nc.dram_tensor("Y1", [NT * P, H], bf16, kind="Internal")
    out_d = nc.dram_tensor("out", [NB * BLK, H], f32, kind="ExternalOutput")

    RELU = mybir.ActivationFunctionType.Relu
    ADD = mybir.AluOpType.add

    with tile.TileContext(nc) as tc:
        # PSUM: 8 banks of 2KB/partition. y1p4/ppre4 are full banks;
        # pagg/u128 round up to one bank per slot. 2+2+2+2 = 8.
        with tc.tile_pool(name="const", bufs=1) as cp, \
             tc.tile_pool(name="y1w", bufs=4) as yp, \
             tc.tile_pool(name="work", bufs=3) as wp, \
             tc.tile_pool(name="onep", bufs=3) as op_, \
             tc.tile_pool(name="gat", bufs=3) as gp, \
             tc.tile_pool(name="psA", bufs=2, space="PSUM") as psA, \
             tc.tile_pool(name="psP", bufs=2, space="PSUM") as psP, \
             tc.tile_pool(name="psG", bufs=2, space="PSUM") as psG, \
             tc.tile_pool(name="psU", bufs=2, space="PSUM") as psU:

            nc.gpsimd.load_library(library_config.mlp)

            def load_const(t, name):
                tl = cp.tile(list(t.shape), t.dtype, tag=name)
                nc.sync.dma_start(out=tl[:], in_=t[:])
                return tl

            w1 = load_const(W1, "w1")
            w2 = load_const(W2, "w2")
            wu1 = load_const(Wu1, "wu1")
            wu2 = load_const(Wu2, "wu2")
            wres = load_const(Wres, "wres")
            bu = load_const(bupd, "bu")
            idf = load_const(identf_d, "identf")
            idxA = load_const(idx_d, "idxA")
            xtc = load_const(xTc, "xtc")

            # ---------- phase Y1: Y1 = x @ W1 (bf16 -> DRAM), 4-wide ----------
            i = 0
            while i < NT:
                gsz = min(4, NT - i)
                wlast = min(P, N - (i + gsz - 1) * P)
                cols = (gsz - 1) * P + wlast
                xt4 = yp.tile([P, 4 * P], bf16, tag="xt4")
                nc.sync.dma_start(out=xt4[:, 0:cols], in_=xT[:, i * P:i * P + cols])
                y1p4 = psA.tile([P, 4 * P], f32, space="PSUM", tag="y1p4")
                for k in range(gsz):
                    wk = P if k < gsz - 1 else wlast
                    nc.tensor.matmul(out=y1p4[0:wk, k * P:k * P + H],
                                     lhsT=xt4[:, k * P:k * P + wk], rhs=w1[:],
                                     start=True, stop=True, skip_group_check=True)
                y1s4 = yp.tile([P, 4 * P], bf16, tag="y1s4")
                nc.vector.tensor_copy(out=y1s4[:, 0:gsz * P], in_=y1p4[:, 0:gsz * P])
                if cols == gsz * P:
                    nc.sync.dma_start(
                        out=Y1[i * P:(i + gsz) * P, :].rearrange(
                            "(c p) h -> p c h", p=P),
                        in_=y1s4[:, 0:gsz * P].rearrange("p (c h) -> p c h", h=H))
                else:
                    for k in range(gsz):
                        wk = P if k < gsz - 1 else wlast
                        nc.sync.dma_start(out=Y1[(i + k) * P:(i + k) * P + wk, :],
                                          in_=y1s4[0:wk, k * P:(k + 1) * P])
                i += gsz

            # ---------- phase E + U per block ----------
            for b in range(NB):
                qn = (b % 4) if QROT else 0
                y1g = gp.tile([P, T * H], bf16, tag="y1g")
                nc.gpsimd.dma_gather(
                    out_ap=y1g[:, 0:8 * H].rearrange("p (c h) -> p c h", h=H),
                    in_ap=Y1[0:NSPLIT, :],
                    idxs_ap=idxA[:, b * 2 * IW:b * 2 * IW + IW],
                    num_idxs=HCAP, num_idxs_reg=HCAP, elem_size=H, queue_num=qn)
                nc.gpsimd.dma_gather(
                    out_ap=y1g[:, 8 * H:16 * H].rearrange("p (c h) -> p c h", h=H),
                    in_ap=Y1[NSPLIT:2 * NSPLIT, :],
                    idxs_ap=idxA[:, b * 2 * IW + IW:(b + 1) * 2 * IW],
                    num_idxs=HCAP, num_idxs_reg=HCAP, elem_size=H, queue_num=qn)

                btT8 = op_.tile([P, T * P], fp8, tag="btT8")
                nc.sync.dma_start(out=btT8[:], in_=btT8_d[:, b * T * P:(b + 1) * T * P])
                bt8 = op_.tile([P, T * BLK], fp8, tag="bt8")
                nc.sync.dma_start(out=bt8[:], in_=bt8_d[:, b * T * BLK:(b + 1) * T * BLK])

                # caug = [x_blk @ W2 ; b_msg ; w3]
                pc = psU.tile([P, P], f32, space="PSUM", tag="u128")
                nc.tensor.matmul(out=pc[0:BLK, :],
                                 lhsT=xtc[:, b * BLK:(b + 1) * BLK],
                                 rhs=w2[:], start=True, stop=True)
                caug = wp.tile([P, H], bf16, tag="caug")
                nc.vector.tensor_copy(out=caug[0:BLK, :], in_=pc[0:BLK, :])
                nc.sync.dma_start(out=caug[BLK:P, :], in_=c2_d[:])

                pre = wp.tile([P, T * H], bf16, tag="pre")
                for i4 in range(4):
                    ppre4 = psP.tile([P, 4 * H], f32, space="PSUM", tag="ppre4")
                    for k in range(4):
                        t = i4 * 4 + k
                        nc.tensor.matmul(out=ppre4[:, k * H:(k + 1) * H],
                                         lhsT=btT8[:, t * P:(t + 1) * P],
                                         rhs=caug[:], start=True, stop=True)
                    nc.vector.scalar_tensor_tensor(
                        out=pre[:, i4 * 4 * H:(i4 + 1) * 4 * H], in0=ppre4[:],
                        scalar=0.0, in1=y1g[:, i4 * 4 * H:(i4 + 1) * 4 * H],
                        op0=ADD, op1=ADD)
                msgb = wp.tile([P, T * H], bf16, tag="msgb")
                nc.scalar.activation(out=msgb[:], in_=pre[:], func=RELU)

                pagg = psG.tile([P, BLK], f32, space="PSUM", tag="pagg")
                for t in range(T):
                    nc.tensor.matmul(out=pagg[:, 0:BLK],
                                     lhsT=msgb[:, t * H:(t + 1) * H],
                                     rhs=bt8[:, t * BLK:(t + 1) * BLK],
                                     start=(t == 0), stop=(t == T - 1))

                # ----- node update for block b ([H, node] orientation) -----
                aggT = wp.tile([H, BLK], bf16, tag="aggT")
                nc.vector.tensor_copy(out=aggT[:], in_=pagg[:, 0:BLK])
                xtb = xtc[:, b * BLK:(b + 1) * BLK]

                pupd = psU.tile([P, P], f32, space="PSUM", tag="u128")
                nc.tensor.matmul(out=pupd[:, 0:BLK], lhsT=wu1[:], rhs=xtb,
                                 start=True, stop=False)
                nc.tensor.matmul(out=pupd[:, 0:BLK], lhsT=wu2[:], rhs=aggT[:],
                                 start=False, stop=True)
                rel = wp.tile([H, BLK], bf16, tag="rel")
                nc.scalar.activation(out=rel[:], in_=pupd[:, 0:BLK], func=RELU,
                                     bias=bu[:])
                pres = psU.tile([P, P], f32, space="PSUM", tag="u128")
                nc.tensor.matmul(out=pres[:, 0:BLK], lhsT=wres[:], rhs=xtb,
                                 start=True, stop=True)
                outT = wp.tile([H, BLK], f32, tag="outT")
                nc.vector.tensor_tensor(out=outT[:], in0=pres[:, 0:BLK],
                                        in1=rel[:], op=ADD)
                ptr = psU.tile([P, P], f32, space="PSUM", tag="u128")
                nc.tensor.matmul(out=ptr[0:BLK, :], lhsT=outT[:], rhs=idf[:],
                                 is_transpose=True)
                outs = wp.tile([BLK, H], f32, tag="outs")
                nc.scalar.copy(out=outs[:], in_=ptr[0:BLK, :])
                nc.sync.dma_start(out=out_d[b * BLK:(b + 1) * BLK, :],
                                  in_=outs[:])

    nc.finalize()
    res = run_bass_kernel_spmd(nc, in_maps, core_ids=list(range(NCORES)),
                               trace=bool(int(os.environ.get("K_TRACE", "0"))))
    return res


def kernel(node_embed, edge_dist, edge_index, W_res, W_msg, b_msg, W_upd, b_upd):
    from concourse import mybir
    bf = ml_dtypes.bfloat16
    f8 = mybir.dt.np(mybir.dt.float8e4)
    node_embed = np.asarray(node_embed, dtype=np.float32)
    edge_dist = np.asarray(edge_dist, dtype=np.float32).reshape(-1)
    row = np.asarray(edge_index[0], dtype=np.int64).astype(np.int32)
    col = np.asarray(edge_index[1], dtype=np.int64).astype(np.int32)
    W_res = np.asarray(W_res, dtype=np.float32)
    W_msg = np.asarray(W_msg, dtype=np.float32)
    b_msg = np.asarray(b_msg, dtype=np.float32)
    W_upd = np.asarray(W_upd, dtype=np.float32)
    b_upd = np.asarray(b_upd, dtype=np.float32)

    order = np.argsort(col, kind="stable")
    scol = col[order]
    srow = row[order]
    sdist = edge_dist[order]

    # per-core greedy blocks: <=BLK nodes, <=HCAP low and <=HCAP high edges
    core_blocks = []
    for core in range(NCORES):
        n0 = core * NODES_PER_CORE
        n1 = min(n0 + NODES_PER_CORE, N)
        blocks = []
        v = n0
        e0 = int(np.searchsorted(scol, v))
        while v < n1:
            vmax = min(v + BLK, n1)
            emax = int(np.searchsorted(scol, vmax))
            lo_cnt = int((srow[e0:emax] < NSPLIT).sum())
            hi_cnt = (emax - e0) - lo_cnt
            vend = vmax
            e1 = emax
            if lo_cnt > HCAP or hi_cnt > HCAP:
                while True:
                    vend_try = v + max(1, (vend - v) * 9 // 10)
                    if vend_try >= vend:
                        vend_try = vend - 1
                    vend = max(v + 1, vend_try)
                    e1 = int(np.searchsorted(scol, vend))
                    lo_cnt = int((srow[e0:e1] < NSPLIT).sum())
                    hi_cnt = (e1 - e0) - lo_cnt
                    if (lo_cnt <= HCAP and hi_cnt <= HCAP) or vend == v + 1:
                        break
            blocks.append((v, vend, e0, e1))
            v = vend
            e0 = e1
        core_blocks.append(blocks)

    NB = max(len(b) for b in core_blocks)
    T = TB
    IW = HCAP // 16
    idxq = np.zeros((NCORES, P, NB * 2 * IW), np.int16)
    btT8 = np.zeros((NCORES, P, NB * T * P), np.float32)
    bt8 = np.zeros((NCORES, P, NB * T * BLK), np.float32)
    xTc = np.zeros((NCORES, C, NB * BLK), np.float32)

    for core in range(NCORES):
        for b, (v0, v1, e0, e1) in enumerate(core_blocks[core]):
            br = srow[e0:e1]
            bc = scol[e0:e1]
            bd = sdist[e0:e1]
            lo_mask = br < NSPLIT
            for half, mask in ((0, lo_mask), (1, ~lo_mask)):
                r = br[mask]
                sub = np.argsort(r, kind="stable")
                r = r[sub]
                c = bc[mask][sub]
                d = bd[mask][sub]
                cnt = len(r)
                assert cnt <= HCAP
                s = np.arange(cnt) + half * HCAP   # block-local slots
                tt, pp_ = s // P, s % P
                k = (c - v0).astype(np.int64)
                # btT8[kslot, tile*P + e] one-hot + valid row + dist row
                btT8[core, k, b * T * P + tt * P + pp_] = 1.0
                btT8[core, BLK, b * T * P + tt * P + pp_] = 1.0        # valid
                btT8[core, BLK + 1, b * T * P + tt * P + pp_] = d      # dist
                # bt8[e, tile*BLK + kslot] one-hot
                bt8[core, pp_, b * T * BLK + tt * BLK + k] = 1.0
                # gather indices
                wrapped_pos = b * 2 * IW + half * IW
                seq = np.zeros(HCAP, np.int64)
                seq[0:cnt] = r - half * NSPLIT
                w2_ = seq.reshape(IW, 16).T.astype(np.int16)
                idxq[core, :, wrapped_pos:wrapped_pos + IW] = np.tile(w2_, (8, 1))
            xTc[core, :, b * BLK:b * BLK + (v1 - v0)] = node_embed[v0:v1].T

    c2 = np.stack([b_msg, W_msg[2 * C]])
    consts = {
        "xT": np.ascontiguousarray(node_embed.T).astype(bf),
        "W1": W_msg[0:C].astype(bf),
        "W2": W_msg[C:2 * C].astype(bf),
        "Wu1": W_upd[0:C].astype(bf),
        "Wu2": W_upd[C:C + H].astype(bf),
        "Wres": W_res.astype(bf),
        "bupd": b_upd.reshape(H, 1),
        "c2": c2.astype(bf),
        "identf": np.eye(P, dtype=np.float32),
    }
    in_maps = []
    for core in range(NCORES):
        m = {"xTc": xTc[core].astype(bf), "idxq": idxq[core],
             "btT8": btT8[core].astype(f8), "bt8": bt8[core].astype(f8)}
        m.update(consts)
        in_maps.append(m)

    res = _build_and_run(in_maps, NB)
    kernel._last_result = res

    out = np.empty((N, H), np.float32)
    for core in range(NCORES):
        o = res.results[core]["out"]
        for b, (v0, v1, _, _) in enumerate(core_blocks[core]):
            out[v0:v1] = o[b * BLK:b * BLK + (v1 - v0)]
    return out


# revision 4
# speedup vs baseline: 1.2126x; 1.0124x over previous
"""Trainium2 Bass kernel for EquivariantMPLayer (GNN message passing), v3.

  msg_repr = [x[row], x[col], edge_dist]            # [E, 2C+1]
  messages = relu(msg_repr @ W_msg + b_msg)         # [E, H]
  aggr     = segment_sum(messages, col, N)          # [N, H]
  out      = x @ W_res + relu([x, aggr] @ W_upd + b_upd)

Strategy (8 NeuronCores, SPMD single program):
  * Host: sort edges by col; shard cores by contiguous node ranges so each
    core's local segment-sum is complete for its node slice. Blocks of
    <=126 nodes; edges split into <=1024 low-row (<25000) and <=1024
    high-row slots; 16 tiles of 128 edges.
  * Phase Y1 (replicated): Y1 = x @ W1 (bf16 -> Internal DRAM), batched 4
    tiles per DVE/DMA op. b_msg rides in caug row 126 against the one-hot
    "valid" row, so no bias pass is needed.
  * Phase E per block: two 1024-row SWDGE dma_gathers of Y1 (row-side
    term; >1024 idxs per instruction deadlocks, and descgen costs ~9ns/row
    on the Q7 so the gathers dominate gpsimd time; queue_num rotation
    spreads them across Q7 core pairs when enabled). One-hot matrices come
    from the host in fp8 (btT8 includes valid row 126 and dist row 127),
    removing all per-tile is_equal/transpose/copyout work. Per 4-tile
    group: 4 matmuls btT8^T @ caug + one fused DVE (ppre + y1g); one ACT
    relu per block; per tile one aggregation matmul msg^T @ bt8
    accumulating aggT [H, node] in PSUM.
  * Node update per block in [H, node] orientation; PE transpose for the
    output write.
"""
import numpy as np
import ml_dtypes
import os

N = 50000
E = 800000
C = 128
H = 128
NCORES = 8
BLK = 126                    # max nodes per block
TB = 16                      # tiles per block
HCAP = 1024                  # per-block capacity of low/high-row halves
NSPLIT = 25000               # row id splitting low/high gather tables
NODES_PER_CORE = 6300
NT = (N + 127) // 128        # Y1 phase tiles (391)
P = 128
QROT = int(os.environ.get("K_QROT", "1"))   # rotate gather queue_num 0..3


def _build_and_run(in_maps, NB):
    import concourse.bacc as bacc
    import concourse.tile as tile
    from concourse import bass, library_config, mybir
    from concourse.bass_utils import run_bass_kernel_spmd

    f32 = mybir.dt.float32
    bf16 = mybir.dt.bfloat16
    fp8 = mybir.dt.float8e4
    i16 = mybir.dt.int16
    T = TB
    IW = HCAP // 16          # idx columns per gather instruction (64)

    nc = bacc.Bacc("TRN2", num_swdge_queues=4 if QROT else 1,
                   dynamic_dma_scratch_size=int(os.environ.get("K_SCRATCH", "65536")))
    nc.cache_partition_id()

    xT = nc.dram_tensor("xT", [C, N], bf16, kind="ExternalInput")
    xTc = nc.dram_tensor("xTc", [C, NB * BLK], bf16, kind="ExternalInput")
    idx_d = nc.dram_tensor("idxq", [P, NB * 2 * IW], i16, kind="ExternalInput")
    btT8_d = nc.dram_tensor("btT8", [P, NB * T * P], fp8, kind="ExternalInput")
    bt8_d = nc.dram_tensor("bt8", [P, NB * T * BLK], fp8, kind="ExternalInput")
    W1 = nc.dram_tensor("W1", [C, H], bf16, kind="ExternalInput")
    W2 = nc.dram_tensor("W2", [C, H], bf16, kind="ExternalInput")
    Wu1 = nc.dram_tensor("Wu1", [C, H], bf16, kind="ExternalInput")
    Wu2 = nc.dram_tensor("Wu2", [H, H], bf16, kind="ExternalInput")
    Wres = nc.dram_tensor("Wres", [C, H], bf16, kind="ExternalInput")
    bupd = nc.dram_tensor("bupd", [H, 1], f32, kind="ExternalInput")
    c2_d = nc.dram_tensor("c2", [2, H], bf16, kind="ExternalInput")
    identf_d = nc.dram_tensor("identf", [P, P], f32, kind="ExternalInput")
    Y1 = nc.dram_tensor("Y1", [NT * P, H], bf16, kind="Internal")
    out_d = nc.dram_tensor("out", [NB * BLK, H], f32, kind="ExternalOutput")

    RELU = mybir.ActivationFunctionType.Relu
    ADD = mybir.AluOpType.add

    with tile.TileContext(nc) as tc:
        # PSUM: 8 banks of 2KB/partition. y1p4/ppre4 are full banks;
        # pagg/u128 round up to one bank per slot. 2+2+2+2 = 8.
        with tc.tile_pool(name="const", bufs=1) as cp, \
             tc.tile_pool(name="y1w", bufs=4) as yp, \
             tc.tile_pool(name="work", bufs=3) as wp, \
             tc.tile_pool(name="onep", bufs=3) as op_, \
             tc.tile_pool(name="gat", bufs=3) as gp, \
             tc.tile_pool(name="psA", bufs=2, space="PSUM") as psA, \
             tc.tile_pool(name="psP", bufs=2, space="PSUM") as psP, \
             tc.tile_pool(name="psG", bufs=2, space="PSUM") as psG, \
             tc.tile_pool(name="psU", bufs=2, space="PSUM") as psU:

            nc.gpsimd.load_library(library_config.mlp)

            def load_const(t, name):
                tl = cp.tile(list(t.shape), t.dtype, tag=name)
                nc.sync.dma_start(out=tl[:], in_=t[:])
                return tl

            w1 = load_const(W1, "w1")
            w2 = load_const(W2, "w2")
            wu1 = load_const(Wu1, "wu1")
            wu2 = load_const(Wu2, "wu2")
            wres = load_const(Wres, "wres")
            bu = load_const(bupd, "bu")
            idf = load_const(identf_d, "identf")
            idxA = load_const(idx_d, "idxA")
            xtc = load_const(xTc, "xtc")

            # ---------- phase Y1: Y1 = x @ W1 (bf16 -> DRAM), 4-wide ----------
            i = 0
            while i < NT:
                gsz = min(4, NT - i)
                wlast = min(P, N - (i + gsz - 1) * P)
                cols = (gsz - 1) * P + wlast
                xt4 = yp.tile([P, 4 * P], bf16, tag="xt4")
                nc.sync.dma_start(out=xt4[:, 0:cols], in_=xT[:, i * P:i * P + cols])
                y1p4 = psA.tile([P, 4 * P], f32, space="PSUM", tag="y1p4")
                for k in range(gsz):
                    wk = P if k < gsz - 1 else wlast
                    nc.tensor.matmul(out=y1p4[0:wk, k * P:k * P + H],
                                     lhsT=xt4[:, k * P:k * P + wk], rhs=w1[:],
                                     start=True, stop=True, skip_group_check=True)
                y1s4 = yp.tile([P, 4 * P], bf16, tag="y1s4")
                nc.vector.tensor_copy(out=y1s4[:, 0:gsz * P], in_=y1p4[:, 0:gsz * P])
                if cols == gsz * P:
                    nc.sync.dma_start(
                        out=Y1[i * P:(i + gsz) * P, :].rearrange(
                            "(c p) h -> p c h", p=P),
                        in_=y1s4[:, 0:gsz * P].rearrange("p (c h) -> p c h", h=H))
                else:
                    for k in range(gsz):
                        wk = P if k < gsz - 1 else wlast
                        nc.sync.dma_start(out=Y1[(i + k) * P:(i + k) * P + wk, :],
                                          in_=y1s4[0:wk, k * P:(k + 1) * P])
                i += gsz

            # ---------- phase E + U per block ----------
            for b in range(NB):
                qn = (b % 4) if QROT else 0
                y1g = gp.tile([P, T * H], bf16, tag="y1g")
                nc.gpsimd.dma_gather(
                    out_ap=y1g[:, 0:8 * H].rearrange("p (c h) -> p c h", h=H),
                    in_ap=Y1[0:NSPLIT, :],
                    idxs_ap=idxA[:, b * 2 * IW:b * 2 * IW + IW],
                    num_idxs=HCAP, num_idxs_reg=HCAP, elem_size=H, queue_num=qn)
                nc.gpsimd.dma_gather(
                    out_ap=y1g[:, 8 * H:16 * H].rearrange("p (c h) -> p c h", h=H),
                    in_ap=Y1[NSPLIT:2 * NSPLIT, :],
                    idxs_ap=idxA[:, b * 2 * IW + IW:(b + 1) * 2 * IW],
                    num_idxs=HCAP, num_idxs_reg=HCAP, elem_size=H, queue_num=qn)

                btT8 = op_.tile([P, T * P], fp8, tag="btT8")
                nc.sync.dma_start(out=btT8[:], in_=btT8_d[:, b * T * P:(b + 1) * T * P])
                bt8 = op_.tile([P, T * BLK], fp8, tag="bt8")
                nc.sync.dma_start(out=bt8[:], in_=bt8_d[:, b * T * BLK:(b + 1) * T * BLK])

                # caug = [x_blk @ W2 ; b_msg ; w3]
                pc = psU.tile([P, P], f32, space="PSUM", tag="u128")
                nc.tensor.matmul(out=pc[0:BLK, :],
                                 lhsT=xtc[:, b * BLK:(b + 1) * BLK],
                                 rhs=w2[:], start=True, stop=True)
                caug = wp.tile([P, H], bf16, tag="caug")
                nc.vector.tensor_copy(out=caug[0:BLK, :], in_=pc[0:BLK, :])
                nc.sync.dma_start(out=caug[BLK:P, :], in_=c2_d[:])

                pre = wp.tile([P, T * H], bf16, tag="pre")
                msgb = wp.tile([P, T * H], bf16, tag="msgb")
                for i4 in range(4):
                    ppre4 = psP.tile([P, 4 * H], f32, space="PSUM", tag="ppre4")
                    for k in range(4):
                        t = i4 * 4 + k
                        nc.tensor.matmul(out=ppre4[:, k * H:(k + 1) * H],
                                         lhsT=btT8[:, t * P:(t + 1) * P],
                                         rhs=caug[:], start=True, stop=True)
                    nc.vector.scalar_tensor_tensor(
                        out=pre[:, i4 * 4 * H:(i4 + 1) * 4 * H], in0=ppre4[:],
                        scalar=0.0, in1=y1g[:, i4 * 4 * H:(i4 + 1) * 4 * H],
                        op0=ADD, op1=ADD)
                    nc.scalar.activation(out=msgb[:, i4 * 4 * H:(i4 + 1) * 4 * H],
                                         in_=pre[:, i4 * 4 * H:(i4 + 1) * 4 * H],
                                         func=RELU)

                pagg = psG.tile([P, BLK], f32, space="PSUM", tag="pagg")
                for t in range(T):
                    nc.tensor.matmul(out=pagg[:, 0:BLK],
                                     lhsT=msgb[:, t * H:(t + 1) * H],
                                     rhs=bt8[:, t * BLK:(t + 1) * BLK],
                                     start=(t == 0), stop=(t == T - 1))

                # ----- node update for block b ([H, node] orientation) -----
                aggT = wp.tile([H, BLK], bf16, tag="aggT")
                nc.vector.tensor_copy(out=aggT[:], in_=pagg[:, 0:BLK])
                xtb = xtc[:, b * BLK:(b + 1) * BLK]

                pupd = psU.tile([P, P], f32, space="PSUM", tag="u128")
                nc.tensor.matmul(out=pupd[:, 0:BLK], lhsT=wu1[:], rhs=xtb,
                                 start=True, stop=False)
                nc.tensor.matmul(out=pupd[:, 0:BLK], lhsT=wu2[:], rhs=aggT[:],
                                 start=False, stop=True)
                rel = wp.tile([H, BLK], bf16, tag="rel")
                nc.scalar.activation(out=rel[:], in_=pupd[:, 0:BLK], func=RELU,
                                     bias=bu[:])
                pres = psU.tile([P, P], f32, space="PSUM", tag="u128")
                nc.tensor.matmul(out=pres[:, 0:BLK], lhsT=wres[:], rhs=xtb,
                                 start=True, stop=True)
                outT = wp.tile([H, BLK], f32, tag="outT")
                nc.vector.tensor_tensor(out=outT[:], in0=pres[:, 0:BLK],
                                        in1=rel[:], op=ADD)
                ptr = psU.tile([P, P], f32, space="PSUM", tag="u128")
                nc.tensor.matmul(out=ptr[0:BLK, :], lhsT=outT[:], rhs=idf[:],
                                 is_transpose=True)
                outs = wp.tile([BLK, H], f32, tag="outs")
                nc.scalar.copy(out=outs[:], in_=ptr[0:BLK, :])
                nc.sync.dma_start(out=out_d[b * BLK:(b + 1) * BLK, :],
                                  in_=outs[:])

    nc.finalize()
    res = run_bass_kernel_spmd(nc, in_maps, core_ids=list(range(NCORES)),
                               trace=bool(int(os.environ.get("K_TRACE", "0"))))
    return res


def kernel(node_embed, edge_dist, edge_index, W_res, W_msg, b_msg, W_upd, b_upd):
    from concourse import mybir
    bf = ml_dtypes.bfloat16
    f8 = mybir.dt.np(mybir.dt.float8e4)
    node_embed = np.asarray(node_embed, dtype=np.float32)
    edge_dist = np.asarray(edge_dist, dtype=np.float32).reshape(-1)
    row = np.asarray(edge_index[0], dtype=np.int64).astype(np.int32)
    col = np.asarray(edge_index[1], dtype=np.int64).astype(np.int32)
    W_res = np.asarray(W_res, dtype=np.float32)
    W_msg = np.asarray(W_msg, dtype=np.float32)
    b_msg = np.asarray(b_msg, dtype=np.float32)
    W_upd = np.asarray(W_upd, dtype=np.float32)
    b_upd = np.asarray(b_upd, dtype=np.float32)

    order = np.argsort(col, kind="stable")
    scol = col[order]
    srow = row[order]
    sdist = edge_dist[order]

    # per-core greedy blocks: <=BLK nodes, <=HCAP low and <=HCAP high edges
    core_blocks = []
    for core in range(NCORES):
        n0 = core * NODES_PER_CORE
        n1 = min(n0 + NODES_PER_CORE, N)
        blocks = []
        v = n0
        e0 = int(np.searchsorted(scol, v))
        while v < n1:
            vmax = min(v + BLK, n1)
            emax = int(np.searchsorted(scol, vmax))
            lo_cnt = int((srow[e0:emax] < NSPLIT).sum())
            hi_cnt = (emax - e0) - lo_cnt
            vend = vmax
            e1 = emax
            if lo_cnt > HCAP or hi_cnt > HCAP:
                while True:
                    vend_try = v + max(1, (vend - v) * 9 // 10)
                    if vend_try >= vend:
                        vend_try = vend - 1
                    vend = max(v + 1, vend_try)
                    e1 = int(np.searchsorted(scol, vend))
                    lo_cnt = int((srow[e0:e1] < NSPLIT).sum())
                    hi_cnt = (e1 - e0) - lo_cnt
                    if (lo_cnt <= HCAP and hi_cnt <= HCAP) or vend == v + 1:
                        break
            blocks.append((v, vend, e0, e1))
            v = vend
            e0 = e1
        core_blocks.append(blocks)

    NB = max(len(b) for b in core_blocks)
    T = TB
    IW = HCAP // 16
    idxq = np.zeros((NCORES, P, NB * 2 * IW), np.int16)
    btT8 = np.zeros((NCORES, P, NB * T * P), np.float32)
    bt8 = np.zeros((NCORES, P, NB * T * BLK), np.float32)
    xTc = np.zeros((NCORES, C, NB * BLK), np.float32)

    for core in range(NCORES):
        for b, (v0, v1, e0, e1) in enumerate(core_blocks[core]):
            br = srow[e0:e1]
            bc = scol[e0:e1]
            bd = sdist[e0:e1]
            lo_mask = br < NSPLIT
            for half, mask in ((0, lo_mask), (1, ~lo_mask)):
                r = br[mask]
                sub = np.argsort(r, kind="stable")
                r = r[sub]
                c = bc[mask][sub]
                d = bd[mask][sub]
                cnt = len(r)
                assert cnt <= HCAP
                s = np.arange(cnt) + half * HCAP   # block-local slots
                tt, pp_ = s // P, s % P
                k = (c - v0).astype(np.int64)
                # btT8[kslot, tile*P + e] one-hot + valid row + dist row
                btT8[core, k, b * T * P + tt * P + pp_] = 1.0
                btT8[core, BLK, b * T * P + tt * P + pp_] = 1.0        # valid
                btT8[core, BLK + 1, b * T * P + tt * P + pp_] = d      # dist
                # bt8[e, tile*BLK + kslot] one-hot
                bt8[core, pp_, b * T * BLK + tt * BLK + k] = 1.0
                # gather indices
                wrapped_pos = b * 2 * IW + half * IW
                seq = np.zeros(HCAP, np.int64)
                seq[0:cnt] = r - half * NSPLIT
                w2_ = seq.reshape(IW, 16).T.astype(np.int16)
                idxq[core, :, wrapped_pos:wrapped_pos + IW] = np.tile(w2_, (8, 1))
            xTc[core, :, b * BLK:b * BLK + (v1 - v0)] = node_embed[v0:v1].T

    c2 = np.stack([b_msg, W_msg[2 * C]])
    consts = {
        "xT": np.ascontiguousarray(node_embed.T).astype(bf),
        "W1": W_msg[0:C].astype(bf),
        "W2": W_msg[C:2 * C].astype(bf),
        "Wu1": W_upd[0:C].astype(bf),
        "Wu2": W_upd[C:C + H].astype(bf),
        "Wres": W_res.astype(bf),
        "bupd": b_upd.reshape(H, 1),
        "c2": c2.astype(bf),
        "identf": np.eye(P, dtype=np.float32),
    }
    in_maps = []
    for core in range(NCORES):
        m = {"xTc": xTc[core].astype(bf), "idxq": idxq[core],
             "btT8": btT8[core].astype(f8), "bt8": bt8[core].astype(f8)}
        m.update(consts)
        in_maps.append(m)

    res = _build_and_run(in_maps, NB)
    kernel._last_result = res

    out = np.empty((N, H), np.float32)
    for core in range(NCORES):
        o = res.results[core]["out"]
        for b, (v0, v1, _, _) in enumerate(core_blocks[core]):
            out[v0:v1] = o[b * BLK:b * BLK + (v1 - v0)]
    return out
